# revision 48
# baseline (speedup 1.0000x reference)
"""Trainium2 Bass kernel for nn_Attention (B=2, S=2048, D=1024, H=16).

Sharding: 8 cores = 2 batches x 4 head-groups (4 heads per core).
Each core computes QKV projection for its batch restricted to its 4 heads,
full (non-causal) attention for those heads, and a partial output
projection over its 256 channels. The host sums the 4 partial outputs per
batch.

v3 design:
 - x transposed on host -> no x PE-transposes on device.
 - one PSUM ring instance for the whole body (slice-level WAR tracking).
 - stage A per 512-token group: qk proj (natural layout) -> rope (DVE
   evens / Pool odds) -> PE transpose (f32r) with ACT PSUM drains; v proj
   drained by Pool into [keytok, head, d|1] layout with a ones column for
   softmax sums.
 - stage B software-pipelined: per key tile kt one 1024-wide exp covers
   both heads of the pair; logits for kt+1 are issued before AV(kt) so
   ACT (the bottleneck: 128 x 1038ns exps) never waits on PE. The
   out-proj (stage C) matmuls of the previous query chunk are inserted
   one-at-a-time into the per-kt PE slack; softmax divide uses a K=1
   broadcast matmul into the Y PSUM bank and partition-shifted Pool
   multiplies.
 - startup DMAs spread across engine queues (xt on SP, wqk on ACT, wv on
   Pool, cos/sin on DVE) so the first matmul starts ~1us in.
"""

import numpy as np

S = 2048
D = 1024
HD = 64
H_LOC = 4  # heads per core
N_CORES = 8
TT = 16  # token tiles of 128
G = 4    # token groups of 512
QC = 4   # query chunks of 512
KT = 16  # key tiles of 128

_CACHED = {}


def build_nc(repeats: int = 1, with_bias: bool = False):
    import concourse.bass as bass_mod
    import concourse.mybir as mybir
    from concourse import bacc
    from concourse.tile import TileContext
    f32 = mybir.dt.float32
    f32r = mybir.dt.float32r
    bf16 = mybir.dt.bfloat16
    Exp = mybir.ActivationFunctionType.Exp

    nc = bacc.Bacc("TRN2", target_bir_lowering=False, debug=False,
                   num_devices=N_CORES)

    xt_d = nc.dram_tensor("xt", [D, S], bf16, kind="ExternalInput")
    cos_d = nc.dram_tensor("cosr", [S, 32], f32, kind="ExternalInput")
    sin_d = nc.dram_tensor("sinr", [S, 32], f32, kind="ExternalInput")
    wqk_d = nc.dram_tensor("wqk", [D, 512], bf16, kind="ExternalInput")
    wv_d = nc.dram_tensor("wv", [D, 256], bf16, kind="ExternalInput")
    wout_d = nc.dram_tensor("wout", [256, D], f32r, kind="ExternalInput")
    ones_d = nc.dram_tensor("ones", [1, 128], f32r, kind="ExternalInput")
    onescol_d = nc.dram_tensor("onescol", [128, 64], f32r,
                               kind="ExternalInput")
    ident_d = nc.dram_tensor("ident", [128, 128], f32r, kind="ExternalInput")
    if with_bias:
        bqk_d = nc.dram_tensor("bqk", [1, 512], bf16, kind="ExternalInput")
        bv_d = nc.dram_tensor("bv", [1, 256], bf16, kind="ExternalInput")
        bout_d = nc.dram_tensor("bout", [1, D], f32r, kind="ExternalInput")
    y_d = nc.dram_tensor("y", [S, D], f32, kind="ExternalOutput")

    with TileContext(nc) as tc:
        with (
            tc.tile_pool(name="const", bufs=1) as cpool,
            tc.tile_pool(name="xin", bufs=2) as xpool,
            tc.tile_pool(name="qkr", bufs=2) as qkrpool,
            tc.tile_pool(name="rtmp", bufs=2) as rtpool,
            tc.tile_pool(name="big", bufs=1) as bigpool,
            tc.tile_pool(name="et", bufs=4) as etpool,
            tc.tile_pool(name="yt", bufs=2) as ypool,
            tc.tile_pool(name="tl", bufs=1) as tailpool,
            tc.tile_pool(name="sml", bufs=1) as spool,
            tc.tile_pool(name="psl", bufs=1, space="PSUM") as psl,
            tc.tile_pool(name="pso", bufs=1, space="PSUM") as pso,
            tc.tile_pool(name="psy", bufs=2, space="PSUM") as psy,
        ):
            # ---- constants / weights (spread across engine DMA queues) ----
            wqk_sb = cpool.tile([128, 8, 512], bf16)
            wv_sb = cpool.tile([128, 8, 256], bf16)
            wout_sb = cpool.tile([128, 2, D], f32r)
            cos_sb = cpool.tile([128, TT, 32], f32)
            sin_sb = cpool.tile([128, TT, 32], f32)
            ones_sb = cpool.tile([1, 128], f32r)
            onescol_sb = cpool.tile([128, 64], f32r)
            ident = cpool.tile([128, 128], f32r)
            if with_bias:
                bqk_sb = cpool.tile([1, 512], bf16)
                bv_sb = cpool.tile([1, 256], bf16)
                bout_sb = cpool.tile([1, D], f32r)
                ones_bf = cpool.tile([1, 128], bf16)

            wqk_r = wqk_d.ap().rearrange("(i p) c -> p i c", p=128)
            for fc in range(8):
                nc.scalar.dma_start(wqk_sb[:, fc, :], wqk_r[:, fc, :])
            nc.gpsimd.dma_start(cos_sb[:], cos_d.ap().rearrange("(t p) c -> p t c", p=128))
            nc.gpsimd.dma_start(sin_sb[:], sin_d.ap().rearrange("(t p) c -> p t c", p=128))
            nc.gpsimd.dma_start(wv_sb[:], wv_d.ap().rearrange("(i p) c -> p i c", p=128))
            nc.gpsimd.dma_start(onescol_sb[:], onescol_d[:])
            nc.gpsimd.dma_start(ones_sb[:], ones_d[:])
            nc.gpsimd.dma_start(ident[:], ident_d[:])
            nc.gpsimd.dma_start(wout_sb[:], wout_d.ap().rearrange("(i p) c -> p i c", p=128))
            if with_bias:
                nc.gpsimd.dma_start(bqk_sb[:], bqk_d[:])
                nc.gpsimd.dma_start(bv_sb[:], bv_d[:])
                nc.gpsimd.dma_start(bout_sb[:], bout_d[:])
                nc.vector.tensor_copy(ones_bf[:], ones_sb[:])

            def bcast8t(ap):
                # [p, t, j] -> [p, t, (bcast 8), j]
                return bass_mod.AP(ap.tensor, ap.offset,
                                   [ap.ap[0], ap.ap[1], [0, 8], ap.ap[2]])

            def body(_iv=None):
                qT = bigpool.tile([128, 2, S], bf16, tag="qT")
                kT = bigpool.tile([128, 2, S], bf16, tag="kT")
                attn = bigpool.tile([128, 2, S], f32r, tag="attn")
                v_sb = bigpool.tile([128, TT, H_LOC, 65], bf16, tag="v")
                # PSUM dep tracking is whole-tensor: separate tensors per role
                LA = psl.tile([128, 2, 512], f32, tag="LA")
                LB = psl.tile([128, 2, 512], f32, tag="LB")
                nc.vector.tensor_copy(
                    v_sb[:, :, :, 64:65],
                    onescol_sb[:].rearrange("p (t h o) -> p t h o", h=H_LOC, o=1))

                # ================= stage A: projections =================
                for g in range(G):
                    xt_g = xpool.tile([128, 8, 512], bf16, tag="xt")
                    xt_r = xt_d[:, g * 512:(g + 1) * 512].rearrange(
                        "(i p) s -> p i s", p=128)
                    if g == 0:
                        for fc in range(8):
                            nc.sync.dma_start(xt_g[:, fc, :], xt_r[:, fc, :])
                    else:
                        nc.sync.dma_start(xt_g[:], xt_r)

                    O_a = pso.tile([128, 2, 512], f32, tag="O", name=f"Oa{g}")
                    qk_sb = tailpool.tile([128, 4, 512], f32, tag="qks",
                                          name=f"qks{g}")
                    qk_r = qkrpool.tile([128, 4, 512], f32r, tag="qkr")

                    def rope_pair(pr):
                        # paired ACT drain + rope for tis (2*pr, 2*pr+1)
                        tt = g * 4 + 2 * pr
                        ts = slice(2 * pr, 2 * pr + 2)
                        nc.scalar.copy(qk_sb[:, ts, :], (LA, LB)[pr][:, :, :])
                        cosp = bcast8t(cos_sb[:, tt:tt + 2, :])
                        sinp = bcast8t(sin_sb[:, tt:tt + 2, :])
                        srcr = qk_sb[:, ts, :].rearrange(
                            "p t (g j two) -> p two t g j", g=8, j=32)
                        dstr = qk_r[:, ts, :].rearrange(
                            "p t (g pm j) -> p pm t g j", pm=2, j=32)
                        ev, od = srcr[:, 0], srcr[:, 1]
                        t1 = rtpool.tile([128, 2, 8, 32], f32, tag="t1")
                        t2 = rtpool.tile([128, 2, 8, 32], f32, tag="t2")
                        nc.vector.tensor_mul(t1[:], od, sinp)
                        nc.vector.tensor_mul(dstr[:, 0], ev, cosp)
                        nc.vector.tensor_sub(dstr[:, 0], dstr[:, 0], t1[:])
                        nc.vector.tensor_mul(t2[:], ev, sinp)
                        nc.vector.tensor_mul(dstr[:, 1], od, cosp)
                        nc.vector.tensor_add(dstr[:, 1], dstr[:, 1], t2[:])

                    # all four qk projections first so both rope chains
                    # overlap the v projections
                    for ti in range(4):
                        ps_qk = (LA, LB)[ti // 2][:, ti % 2, :]
                        for fc in range(8):
                            nc.tensor.matmul(
                                ps_qk, xt_g[:, fc, ti * 128:(ti + 1) * 128],
                                wqk_sb[:, fc, :],
                                start=(fc == 0), stop=(not with_bias and fc == 7))
                        if with_bias:
                            nc.tensor.matmul(ps_qk, ones_bf[0:1, 0:128], bqk_sb[:],
                                             start=False, stop=True)
                        if ti % 2 == 1:
                            rope_pair(ti // 2)

                    for ti in range(4):
                        tt = g * 4 + ti
                        ps_v = O_a[:, ti % 2, 0:256]
                        for fc in range(8):
                            nc.tensor.matmul(
                                ps_v, xt_g[:, fc, ti * 128:(ti + 1) * 128],
                                wv_sb[:, fc, :],
                                start=(fc == 0), stop=(not with_bias and fc == 7))
                        if with_bias:
                            nc.tensor.matmul(ps_v, ones_bf[0:1, 0:128], bv_sb[:],
                                             start=False, stop=True)
                        if ti % 2 == 1:
                            nc.vector.tensor_copy(
                                v_sb[:, tt - 1:tt + 1, :, 0:64],
                                O_a[:, :, 0:256].rearrange(
                                    "p t (h d) -> p t h d", h=H_LOC))

                    # ---- transpose roped qk into qT/kT (paired ACT drains) ----
                    for cc in range(4):
                        tgt = (LA, LB)[cc // 2]
                        ps_t = tgt[:, cc % 2, :].bitcast(f32r)
                        for ti in range(4):
                            nc.tensor.transpose(
                                ps_t[:, ti * 128:(ti + 1) * 128],
                                qk_r[:, ti, cc * 128:(cc + 1) * 128],
                                ident[:])
                        if cc % 2 == 1:
                            dstbuf = qT if cc < 2 else kT
                            nc.scalar.copy(
                                dstbuf[:, :, g * 512:(g + 1) * 512],
                                tgt[:, :, :].bitcast(f32r))

                # ============ stage B: attention (+ C interleaved) ============
                units = [(qc, hp) for qc in range(QC) for hp in range(2)]

                def logits(qc, hp, kt):
                    tgt = (LA, LB)[kt % 2]
                    for (j, h64) in ((0, 0), (1, 64)):
                        nc.tensor.matmul(
                            tgt[:, j, :],
                            kT[h64:h64 + 64, hp, kt * 128:(kt + 1) * 128],
                            qT[h64:h64 + 64, hp, qc * 512:(qc + 1) * 512],
                            start=True, stop=True)

                def c_unit_mm(qc2, u, half, y_ps):
                    # one matmul of out-proj unit u (token tile ti, chunk ec)
                    ti, ec = divmod(u, 2)
                    tt = qc2 * 4 + ti
                    nc.tensor.matmul(
                        y_ps[:], attn[:, half, tt * 128:(tt + 1) * 128],
                        wout_sb[:, half, ec * 512:(ec + 1) * 512],
                        start=(half == 0),
                        stop=(half == 1 and not with_bias))

                def c_unit_finish(qc2, u, y_ps):
                    ti, ec = divmod(u, 2)
                    tt = qc2 * 4 + ti
                    if with_bias:
                        nc.tensor.matmul(y_ps[:], ones_sb[0:1, 0:128],
                                         bout_sb[0:1, ec * 512:(ec + 1) * 512],
                                         start=False, stop=True)
                    y_t = ypool.tile([128, 512], f32, tag="yt")
                    nc.vector.tensor_copy(y_t[:], y_ps[:])
                    nc.sync.dma_start(
                        y_d[tt * 128:(tt + 1) * 128, ec * 512:(ec + 1) * 512],
                        y_t[:])

                def divide1(ui, qc, hp, O):
                    # Drain O to SBUF immediately (frees O for the next
                    # unit's AV accumulation) and take the reciprocal of
                    # the sums row — all DVE, off the PE critical path.
                    o_sb = spool.tile([65, 2, 512], f32r, tag="osb",
                                      name=f"osb{ui}")
                    nc.vector.tensor_copy(o_sb[:, :, :], O[0:65, :, :])
                    rec = spool.tile([1, 1024], f32r, tag="rec",
                                     name=f"rec{ui}")
                    with nc.allow_low_precision(
                            reason="f32r reciprocal feeds f32r multiply"):
                        nc.vector.reciprocal(rec[0:1, :],
                                             o_sb[64:65, :, :].rearrange(
                                                 "p a b -> p (a b)"))
                    return (ui, qc, hp, o_sb, rec)

                def divide2(ui, qc, hp, o_sb, rec):
                    # K=1 broadcast matmuls (PE, ridden in a later window)
                    # then partition-shifted DVE multiplies into attn.
                    bce = psy.tile([128, 512], f32, tag="Y", name=f"bce{ui}")
                    nc.tensor.matmul(bce[0:64, :], ones_sb[0:1, 0:64],
                                     rec[0:1, 0:512], start=True, stop=True)
                    bco = psy.tile([128, 512], f32, tag="Y", name=f"bco{ui}")
                    nc.tensor.matmul(bco[0:64, :], ones_sb[0:1, 0:64],
                                     rec[0:1, 512:1024], start=True, stop=True)
                    qs = slice(qc * 512, (qc + 1) * 512)
                    nc.vector.tensor_mul(attn[0:64, hp, qs],
                                         o_sb[0:64, 0, :], bce[0:64, :])
                    nc.vector.tensor_mul(attn[64:128, hp, qs],
                                         o_sb[0:64, 1, :], bco[0:64, :])

                # tail half0 out-proj: ridden in the last unit's kt>=10
                # windows (they only need divide(3,0)); drained to SBUF so
                # the tail only runs the hp=1 matmuls + add-drains.
                tail_yt = {}

                def tail_half0(u):
                    ti, ec = divmod(u, 2)
                    tt = (QC - 1) * 4 + ti
                    yp = psy.tile([128, 512], f32, tag="Y", name=f"t0y{u}")
                    nc.tensor.matmul(
                        yp[:], attn[:, 0, tt * 128:(tt + 1) * 128],
                        wout_sb[:, 0, ec * 512:(ec + 1) * 512],
                        start=True, stop=True)
                    y_t = tailpool.tile([128, 512], f32, tag=f"tyt{u}")
                    nc.vector.tensor_copy(y_t[:], yp[:])
                    tail_yt[u] = y_t

                logits(0, 0, 0)  # prologue for the first unit
                pdiv = [None]
                for ui, (qc, hp) in enumerate(units):
                    O = pso.tile([128, 2, 512], f32, tag="O", name=f"O{ui}")
                    # pending out-proj half-matmuls for the previous chunk
                    pend = []
                    if hp == 0 and qc > 0:
                        pend = [(qc - 1, u) for u in range(4)]
                    elif hp == 1 and qc > 0:
                        pend = [(qc - 1, u) for u in range(4, 8)]
                    pend_i = 0
                    cur_y = [None]

                    for kt in range(KT):
                        ering = etpool.tile([128, 2, 512], bf16, tag="er")
                        src = (LA, LB)[kt % 2]
                        nc.scalar.activation(ering[:], src[:, :, :],
                                             Exp, scale=0.125)
                        if kt < KT - 1:
                            logits(qc, hp, kt + 1)
                        elif ui + 1 < len(units):
                            qc2, hp2 = units[ui + 1]
                            logits(qc2, hp2, 0)  # next unit's prologue
                        nc.tensor.matmul(
                            O[0:65, 0, :], v_sb[:, kt, 2 * hp, :],
                            ering[:, 0, :], start=(kt == 0), stop=(kt == KT - 1))
                        nc.tensor.matmul(
                            O[0:65, 1, :], v_sb[:, kt, 2 * hp + 1, :],
                            ering[:, 1, :], start=(kt == 0), stop=(kt == KT - 1))
                        if kt == 1 and pdiv[0] is not None:
                            # previous unit's broadcast+multiply rides here
                            divide2(*pdiv[0])
                            pdiv[0] = None
                        # ride one out-proj matmul in the per-kt PE slack
                        if kt >= 3 and pend_i < 2 * len(pend):
                            qc2, u = pend[pend_i // 2]
                            half = pend_i % 2
                            if half == 0:
                                cur_y[0] = psy.tile([128, 512], f32, tag="Y",
                                                    name=f"ycu{ui}_{pend_i}")
                            c_unit_mm(qc2, u, half, cur_y[0])
                            if half == 1:
                                c_unit_finish(qc2, u, cur_y[0])
                            pend_i += 1
                        elif ui == len(units) - 1 and kt >= 10:
                            tail_half0(kt - 10)

                    pdiv[0] = divide1(ui, qc, hp, O)

                # final unit's divide runs in the tail
                divide2(*pdiv[0])

                # tail: hp=1 half of the last query chunk's out-proj
                for u in range(8):
                    if u not in tail_yt:
                        tail_half0(u)
                for u in range(8):
                    ti, ec = divmod(u, 2)
                    tt = (QC - 1) * 4 + ti
                    yp = psy.tile([128, 512], f32, tag="Y", name=f"t1y{u}")
                    nc.tensor.matmul(
                        yp[:], attn[:, 1, tt * 128:(tt + 1) * 128],
                        wout_sb[:, 1, ec * 512:(ec + 1) * 512],
                        start=True, stop=not with_bias)
                    if with_bias:
                        nc.tensor.matmul(yp[:], ones_sb[0:1, 0:128],
                                         bout_sb[0:1, ec * 512:(ec + 1) * 512],
                                         start=False, stop=True)
                    y_t = tail_yt[u]
                    nc.vector.tensor_add(y_t[:], y_t[:], yp[:])
                    nc.sync.dma_start(
                        y_d[tt * 128:(tt + 1) * 128, ec * 512:(ec + 1) * 512],
                        y_t[:])

            if repeats == 1:
                body()
            else:
                with tc.For_i(0, repeats, 1) as _i:
                    body(_i)

    nc.compile()
    return nc


def _prep_in_maps(x, rope_cos, rope_sin, W_qkv, b_qkv, W_out, b_out,
                  with_bias=False):
    f32 = np.float32
    W3 = np.asarray(W_qkv, dtype=f32).reshape(D, 16, 3, HD)  # [f, head, qkv, d]
    b3 = np.asarray(b_qkv, dtype=f32).reshape(16, 3, HD)
    cos_r = np.ascontiguousarray(np.asarray(rope_cos, dtype=f32))
    sin_r = np.ascontiguousarray(np.asarray(rope_sin, dtype=f32))
    ones = np.ones((1, 128), dtype=f32)
    onescol = np.ones((128, 64), dtype=f32)
    W_out = np.asarray(W_out, dtype=f32)
    b_out = np.asarray(b_out, dtype=f32)
    x = np.asarray(x, dtype=f32)

    in_maps = []
    for c in range(N_CORES):
        b, hg = divmod(c, 4)
        hs = slice(hg * H_LOC, (hg + 1) * H_LOC)
        wq = W3[:, hs, 0, :].reshape(D, 256)
        wk = W3[:, hs, 1, :].reshape(D, 256)
        wv = W3[:, hs, 2, :].reshape(D, 256)
        import ml_dtypes
        m = {
            "xt": np.ascontiguousarray(x[b].T).astype(ml_dtypes.bfloat16),
            "cosr": cos_r, "sinr": sin_r,
            "wqk": np.ascontiguousarray(
                np.concatenate([wq, wk], axis=1)).astype(ml_dtypes.bfloat16),
            "wv": np.ascontiguousarray(wv).astype(ml_dtypes.bfloat16),
            "wout": np.ascontiguousarray(W_out[hg * 256:(hg + 1) * 256, :]),
            "ones": ones, "onescol": onescol,
            "ident": np.eye(128, dtype=f32),
        }
        if with_bias:
            bq = b3[hs, 0, :].reshape(1, 256)
            bk = b3[hs, 1, :].reshape(1, 256)
            m["bqk"] = np.ascontiguousarray(np.concatenate([bq, bk], axis=1))
            m["bv"] = np.ascontiguousarray(b3[hs, 2, :].reshape(1, 256))
            m["bout"] = (np.ascontiguousarray(b_out.reshape(1, D)) if hg == 0
                         else np.zeros((1, D), dtype=f32))
        in_maps.append(m)
    return in_maps


def kernel(x, rope_cos, rope_sin, W_qkv, b_qkv, W_out, b_out):
    from concourse.bass_utils import run_bass_kernel_spmd

    with_bias = bool(np.any(np.asarray(b_qkv)) or np.any(np.asarray(b_out)))
    key = ("nc", with_bias)
    if key not in _CACHED:
        _CACHED[key] = build_nc(1, with_bias=with_bias)
        _CACHED["nc"] = _CACHED[key]  # convenience for test harness
    nc = _CACHED[key]
    in_maps = _prep_in_maps(x, rope_cos, rope_sin, W_qkv, b_qkv, W_out, b_out,
                            with_bias=with_bias)
    res = run_bass_kernel_spmd(nc, in_maps, list(range(N_CORES)))
    B = x.shape[0]
    out = np.zeros((B, S, D), dtype=np.float32)
    for c in range(N_CORES):
        b = c // 4
        out[b] += res.results[c]["y"]
    return out


# revision 49
# speedup vs baseline: 1.0940x; 1.0940x over previous
"""Trainium2 Bass kernel for nn_Attention (B=2, S=2048, D=1024, H=16).

Sharding: 8 cores = 2 batches x 4 head-groups (4 heads per core).
Each core computes QKV projection for its batch restricted to its 4 heads,
full (non-causal) attention for those heads, and a partial output
projection over its 256 channels. The host sums the 4 partial outputs per
batch.

v3 design:
 - x transposed on host -> no x PE-transposes on device.
 - one PSUM ring instance for the whole body (slice-level WAR tracking).
 - stage A per 512-token group: qk proj (natural layout) -> rope (DVE
   evens / Pool odds) -> PE transpose (f32r) with ACT PSUM drains; v proj
   drained by Pool into [keytok, head, d|1] layout with a ones column for
   softmax sums.
 - stage B software-pipelined: per key tile kt one 1024-wide exp covers
   both heads of the pair; logits for kt+1 are issued before AV(kt) so
   ACT (the bottleneck: 128 x 1038ns exps) never waits on PE. The
   out-proj (stage C) matmuls of the previous query chunk are inserted
   one-at-a-time into the per-kt PE slack; softmax divide uses a K=1
   broadcast matmul into the Y PSUM bank and partition-shifted Pool
   multiplies.
 - startup DMAs spread across engine queues (xt on SP, wqk on ACT, wv on
   Pool, cos/sin on DVE) so the first matmul starts ~1us in.
"""

import numpy as np

S = 2048
D = 1024
HD = 64
H_LOC = 4  # heads per core
N_CORES = 8
TT = 16  # token tiles of 128
G = 4    # token groups of 512
QC = 4   # query chunks of 512
KT = 16  # key tiles of 128

_CACHED = {}


def build_nc(repeats: int = 1, with_bias: bool = False):
    import concourse.bass as bass_mod
    import concourse.mybir as mybir
    from concourse import bacc
    from concourse.tile import TileContext
    f32 = mybir.dt.float32
    f32r = mybir.dt.float32r
    bf16 = mybir.dt.bfloat16
    Exp = mybir.ActivationFunctionType.Exp

    nc = bacc.Bacc("TRN2", target_bir_lowering=False, debug=False,
                   num_devices=N_CORES)

    xt_d = nc.dram_tensor("xt", [D, S], bf16, kind="ExternalInput")
    cos_d = nc.dram_tensor("cosr", [S, 32], f32, kind="ExternalInput")
    sin_d = nc.dram_tensor("sinr", [S, 32], f32, kind="ExternalInput")
    wqk_d = nc.dram_tensor("wqk", [D, 512], bf16, kind="ExternalInput")
    wv_d = nc.dram_tensor("wv", [D, 256], bf16, kind="ExternalInput")
    wout_d = nc.dram_tensor("wout", [256, D], f32r, kind="ExternalInput")
    ones_d = nc.dram_tensor("ones", [1, 128], f32r, kind="ExternalInput")
    onescol_d = nc.dram_tensor("onescol", [128, 64], f32r,
                               kind="ExternalInput")
    ident_d = nc.dram_tensor("ident", [128, 128], f32r, kind="ExternalInput")
    if with_bias:
        bqk_d = nc.dram_tensor("bqk", [1, 512], bf16, kind="ExternalInput")
        bv_d = nc.dram_tensor("bv", [1, 256], bf16, kind="ExternalInput")
        bout_d = nc.dram_tensor("bout", [1, D], f32r, kind="ExternalInput")
    y_d = nc.dram_tensor("y", [S, D], bf16, kind="ExternalOutput")

    with TileContext(nc) as tc:
        with (
            tc.tile_pool(name="const", bufs=1) as cpool,
            tc.tile_pool(name="xin", bufs=2) as xpool,
            tc.tile_pool(name="qkr", bufs=2) as qkrpool,
            tc.tile_pool(name="rtmp", bufs=2) as rtpool,
            tc.tile_pool(name="big", bufs=1) as bigpool,
            tc.tile_pool(name="et", bufs=4) as etpool,
            tc.tile_pool(name="yt", bufs=2) as ypool,
            tc.tile_pool(name="tl", bufs=1) as tailpool,
            tc.tile_pool(name="sml", bufs=1) as spool,
            tc.tile_pool(name="psl", bufs=1, space="PSUM") as psl,
            tc.tile_pool(name="pso", bufs=1, space="PSUM") as pso,
            tc.tile_pool(name="psy", bufs=2, space="PSUM") as psy,
        ):
            # ---- constants / weights (spread across engine DMA queues) ----
            wqk_sb = cpool.tile([128, 8, 512], bf16)
            wv_sb = cpool.tile([128, 8, 256], bf16)
            wout_sb = cpool.tile([128, 2, D], f32r)
            cos_sb = cpool.tile([128, TT, 32], f32)
            sin_sb = cpool.tile([128, TT, 32], f32)
            ones_sb = cpool.tile([1, 128], f32r)
            onescol_sb = cpool.tile([128, 64], f32r)
            ident = cpool.tile([128, 128], f32r)
            if with_bias:
                bqk_sb = cpool.tile([1, 512], bf16)
                bv_sb = cpool.tile([1, 256], bf16)
                bout_sb = cpool.tile([1, D], f32r)
                ones_bf = cpool.tile([1, 128], bf16)

            wqk_r = wqk_d.ap().rearrange("(i p) c -> p i c", p=128)
            for fc in range(8):
                nc.scalar.dma_start(wqk_sb[:, fc, :], wqk_r[:, fc, :])
            nc.gpsimd.dma_start(cos_sb[:], cos_d.ap().rearrange("(t p) c -> p t c", p=128))
            nc.gpsimd.dma_start(sin_sb[:], sin_d.ap().rearrange("(t p) c -> p t c", p=128))
            nc.gpsimd.dma_start(wv_sb[:], wv_d.ap().rearrange("(i p) c -> p i c", p=128))
            nc.gpsimd.dma_start(onescol_sb[:], onescol_d[:])
            nc.gpsimd.dma_start(ones_sb[:], ones_d[:])
            nc.gpsimd.dma_start(ident[:], ident_d[:])
            nc.gpsimd.dma_start(wout_sb[:], wout_d.ap().rearrange("(i p) c -> p i c", p=128))
            if with_bias:
                nc.gpsimd.dma_start(bqk_sb[:], bqk_d[:])
                nc.gpsimd.dma_start(bv_sb[:], bv_d[:])
                nc.gpsimd.dma_start(bout_sb[:], bout_d[:])
                nc.vector.tensor_copy(ones_bf[:], ones_sb[:])

            def bcast8t(ap):
                # [p, t, j] -> [p, t, (bcast 8), j]
                return bass_mod.AP(ap.tensor, ap.offset,
                                   [ap.ap[0], ap.ap[1], [0, 8], ap.ap[2]])

            def body(_iv=None):
                qT = bigpool.tile([128, 2, S], bf16, tag="qT")
                kT = bigpool.tile([128, 2, S], bf16, tag="kT")
                attn = bigpool.tile([128, 2, S], f32r, tag="attn")
                v_sb = bigpool.tile([128, TT, H_LOC, 65], bf16, tag="v")
                # PSUM dep tracking is whole-tensor: separate tensors per role
                LA = psl.tile([128, 2, 512], f32, tag="LA")
                LB = psl.tile([128, 2, 512], f32, tag="LB")
                nc.vector.tensor_copy(
                    v_sb[:, :, :, 64:65],
                    onescol_sb[:].rearrange("p (t h o) -> p t h o", h=H_LOC, o=1))

                # ================= stage A: projections =================
                for g in range(G):
                    xt_g = xpool.tile([128, 8, 512], bf16, tag="xt")
                    xt_r = xt_d[:, g * 512:(g + 1) * 512].rearrange(
                        "(i p) s -> p i s", p=128)
                    if g == 0:
                        for fc in range(8):
                            nc.sync.dma_start(xt_g[:, fc, :], xt_r[:, fc, :])
                    else:
                        nc.sync.dma_start(xt_g[:], xt_r)

                    O_a = pso.tile([128, 2, 512], f32, tag="O", name=f"Oa{g}")
                    qk_sb = tailpool.tile([128, 4, 512], f32, tag="qks",
                                          name=f"qks{g}")
                    qk_r = qkrpool.tile([128, 4, 512], f32r, tag="qkr")

                    def rope_pair(pr):
                        # paired ACT drain + rope for tis (2*pr, 2*pr+1)
                        tt = g * 4 + 2 * pr
                        ts = slice(2 * pr, 2 * pr + 2)
                        nc.scalar.copy(qk_sb[:, ts, :], (LA, LB)[pr][:, :, :])
                        cosp = bcast8t(cos_sb[:, tt:tt + 2, :])
                        sinp = bcast8t(sin_sb[:, tt:tt + 2, :])
                        srcr = qk_sb[:, ts, :].rearrange(
                            "p t (g j two) -> p two t g j", g=8, j=32)
                        dstr = qk_r[:, ts, :].rearrange(
                            "p t (g pm j) -> p pm t g j", pm=2, j=32)
                        ev, od = srcr[:, 0], srcr[:, 1]
                        t1 = rtpool.tile([128, 2, 8, 32], f32, tag="t1")
                        t2 = rtpool.tile([128, 2, 8, 32], f32, tag="t2")
                        nc.vector.tensor_mul(t1[:], od, sinp)
                        nc.vector.tensor_mul(dstr[:, 0], ev, cosp)
                        nc.vector.tensor_sub(dstr[:, 0], dstr[:, 0], t1[:])
                        nc.vector.tensor_mul(t2[:], ev, sinp)
                        nc.vector.tensor_mul(dstr[:, 1], od, cosp)
                        nc.vector.tensor_add(dstr[:, 1], dstr[:, 1], t2[:])

                    # all four qk projections first so both rope chains
                    # overlap the v projections
                    for ti in range(4):
                        ps_qk = (LA, LB)[ti // 2][:, ti % 2, :]
                        for fc in range(8):
                            nc.tensor.matmul(
                                ps_qk, xt_g[:, fc, ti * 128:(ti + 1) * 128],
                                wqk_sb[:, fc, :],
                                start=(fc == 0), stop=(not with_bias and fc == 7))
                        if with_bias:
                            nc.tensor.matmul(ps_qk, ones_bf[0:1, 0:128], bqk_sb[:],
                                             start=False, stop=True)
                        if ti % 2 == 1:
                            rope_pair(ti // 2)

                    for ti in range(4):
                        tt = g * 4 + ti
                        ps_v = O_a[:, ti % 2, 0:256]
                        for fc in range(8):
                            nc.tensor.matmul(
                                ps_v, xt_g[:, fc, ti * 128:(ti + 1) * 128],
                                wv_sb[:, fc, :],
                                start=(fc == 0), stop=(not with_bias and fc == 7))
                        if with_bias:
                            nc.tensor.matmul(ps_v, ones_bf[0:1, 0:128], bv_sb[:],
                                             start=False, stop=True)
                        if ti % 2 == 1:
                            nc.vector.tensor_copy(
                                v_sb[:, tt - 1:tt + 1, :, 0:64],
                                O_a[:, :, 0:256].rearrange(
                                    "p t (h d) -> p t h d", h=H_LOC))

                    # ---- transpose roped qk into qT/kT (paired ACT drains) ----
                    for cc in range(4):
                        tgt = (LA, LB)[cc // 2]
                        ps_t = tgt[:, cc % 2, :].bitcast(f32r)
                        for ti in range(4):
                            nc.tensor.transpose(
                                ps_t[:, ti * 128:(ti + 1) * 128],
                                qk_r[:, ti, cc * 128:(cc + 1) * 128],
                                ident[:])
                        if cc % 2 == 1:
                            dstbuf = qT if cc < 2 else kT
                            nc.scalar.copy(
                                dstbuf[:, :, g * 512:(g + 1) * 512],
                                tgt[:, :, :].bitcast(f32r))

                # ============ stage B: attention (+ C interleaved) ============
                units = [(qc, hp) for qc in range(QC) for hp in range(2)]

                def logits(qc, hp, kt):
                    tgt = (LA, LB)[kt % 2]
                    for (j, h64) in ((0, 0), (1, 64)):
                        nc.tensor.matmul(
                            tgt[:, j, :],
                            kT[h64:h64 + 64, hp, kt * 128:(kt + 1) * 128],
                            qT[h64:h64 + 64, hp, qc * 512:(qc + 1) * 512],
                            start=True, stop=True)

                def c_unit_mm(qc2, u, half, y_ps):
                    # one matmul of out-proj unit u (token tile ti, chunk ec)
                    ti, ec = divmod(u, 2)
                    tt = qc2 * 4 + ti
                    nc.tensor.matmul(
                        y_ps[:], attn[:, half, tt * 128:(tt + 1) * 128],
                        wout_sb[:, half, ec * 512:(ec + 1) * 512],
                        start=(half == 0),
                        stop=(half == 1 and not with_bias))

                def c_unit_finish(qc2, u, y_ps):
                    ti, ec = divmod(u, 2)
                    tt = qc2 * 4 + ti
                    if with_bias:
                        nc.tensor.matmul(y_ps[:], ones_sb[0:1, 0:128],
                                         bout_sb[0:1, ec * 512:(ec + 1) * 512],
                                         start=False, stop=True)
                    y_t = ypool.tile([128, 512], bf16, tag="yt")
                    nc.vector.tensor_copy(y_t[:], y_ps[:])
                    nc.sync.dma_start(
                        y_d[tt * 128:(tt + 1) * 128, ec * 512:(ec + 1) * 512],
                        y_t[:])

                def divide1(ui, qc, hp, O):
                    # Drain O to SBUF immediately (frees O for the next
                    # unit's AV accumulation) and take the reciprocal of
                    # the sums row — all DVE, off the PE critical path.
                    o_sb = spool.tile([65, 2, 512], f32r, tag="osb",
                                      name=f"osb{ui}")
                    nc.vector.tensor_copy(o_sb[:, :, :], O[0:65, :, :])
                    rec = spool.tile([1, 1024], f32r, tag="rec",
                                     name=f"rec{ui}")
                    with nc.allow_low_precision(
                            reason="f32r reciprocal feeds f32r multiply"):
                        nc.vector.reciprocal(rec[0:1, :],
                                             o_sb[64:65, :, :].rearrange(
                                                 "p a b -> p (a b)"))
                    return (ui, qc, hp, o_sb, rec)

                def divide2(ui, qc, hp, o_sb, rec):
                    # K=1 broadcast matmuls (PE, ridden in a later window)
                    # then partition-shifted DVE multiplies into attn.
                    bce = psy.tile([128, 512], f32, tag="Y", name=f"bce{ui}")
                    nc.tensor.matmul(bce[0:64, :], ones_sb[0:1, 0:64],
                                     rec[0:1, 0:512], start=True, stop=True)
                    bco = psy.tile([128, 512], f32, tag="Y", name=f"bco{ui}")
                    nc.tensor.matmul(bco[0:64, :], ones_sb[0:1, 0:64],
                                     rec[0:1, 512:1024], start=True, stop=True)
                    qs = slice(qc * 512, (qc + 1) * 512)
                    nc.vector.tensor_mul(attn[0:64, hp, qs],
                                         o_sb[0:64, 0, :], bce[0:64, :])
                    nc.vector.tensor_mul(attn[64:128, hp, qs],
                                         o_sb[0:64, 1, :], bco[0:64, :])

                # tail half0 out-proj: ridden in the last unit's kt>=10
                # windows (they only need divide(3,0)); drained to SBUF so
                # the tail only runs the hp=1 matmuls + add-drains.
                tail_yt = {}

                def tail_half0(u):
                    ti, ec = divmod(u, 2)
                    tt = (QC - 1) * 4 + ti
                    yp = psy.tile([128, 512], f32, tag="Y", name=f"t0y{u}")
                    nc.tensor.matmul(
                        yp[:], attn[:, 0, tt * 128:(tt + 1) * 128],
                        wout_sb[:, 0, ec * 512:(ec + 1) * 512],
                        start=True, stop=True)
                    y_t = tailpool.tile([128, 512], bf16, tag=f"tyt{u}")
                    nc.vector.tensor_copy(y_t[:], yp[:])
                    tail_yt[u] = y_t

                logits(0, 0, 0)  # prologue for the first unit
                pdiv = [None]
                for ui, (qc, hp) in enumerate(units):
                    O = pso.tile([128, 2, 512], f32, tag="O", name=f"O{ui}")
                    # pending out-proj half-matmuls for the previous chunk
                    pend = []
                    if hp == 0 and qc > 0:
                        pend = [(qc - 1, u) for u in range(4)]
                    elif hp == 1 and qc > 0:
                        pend = [(qc - 1, u) for u in range(4, 8)]
                    pend_i = 0
                    cur_y = [None]

                    for kt in range(KT):
                        ering = etpool.tile([128, 2, 512], bf16, tag="er")
                        src = (LA, LB)[kt % 2]
                        nc.scalar.activation(ering[:], src[:, :, :],
                                             Exp, scale=0.125)
                        if kt < KT - 1:
                            logits(qc, hp, kt + 1)
                        elif ui + 1 < len(units):
                            qc2, hp2 = units[ui + 1]
                            logits(qc2, hp2, 0)  # next unit's prologue
                        nc.tensor.matmul(
                            O[0:65, 0, :], v_sb[:, kt, 2 * hp, :],
                            ering[:, 0, :], start=(kt == 0), stop=(kt == KT - 1))
                        nc.tensor.matmul(
                            O[0:65, 1, :], v_sb[:, kt, 2 * hp + 1, :],
                            ering[:, 1, :], start=(kt == 0), stop=(kt == KT - 1))
                        if kt == 1 and pdiv[0] is not None:
                            # previous unit's broadcast+multiply rides here
                            divide2(*pdiv[0])
                            pdiv[0] = None
                        # ride one out-proj matmul in the per-kt PE slack
                        if kt >= 3 and pend_i < 2 * len(pend):
                            qc2, u = pend[pend_i // 2]
                            half = pend_i % 2
                            if half == 0:
                                cur_y[0] = psy.tile([128, 512], f32, tag="Y",
                                                    name=f"ycu{ui}_{pend_i}")
                            c_unit_mm(qc2, u, half, cur_y[0])
                            if half == 1:
                                c_unit_finish(qc2, u, cur_y[0])
                            pend_i += 1
                        elif ui == len(units) - 1 and kt >= 10:
                            tail_half0(kt - 10)

                    pdiv[0] = divide1(ui, qc, hp, O)

                # final unit's divide runs in the tail
                divide2(*pdiv[0])

                # tail: hp=1 half of the last query chunk's out-proj
                for u in range(8):
                    if u not in tail_yt:
                        tail_half0(u)
                for u in range(8):
                    ti, ec = divmod(u, 2)
                    tt = (QC - 1) * 4 + ti
                    yp = psy.tile([128, 512], f32, tag="Y", name=f"t1y{u}")
                    nc.tensor.matmul(
                        yp[:], attn[:, 1, tt * 128:(tt + 1) * 128],
                        wout_sb[:, 1, ec * 512:(ec + 1) * 512],
                        start=True, stop=not with_bias)
                    if with_bias:
                        nc.tensor.matmul(yp[:], ones_sb[0:1, 0:128],
                                         bout_sb[0:1, ec * 512:(ec + 1) * 512],
                                         start=False, stop=True)
                    y_t = tail_yt[u]
                    nc.vector.tensor_add(y_t[:], y_t[:], yp[:])
                    nc.sync.dma_start(
                        y_d[tt * 128:(tt + 1) * 128, ec * 512:(ec + 1) * 512],
                        y_t[:])

            if repeats == 1:
                body()
            else:
                with tc.For_i(0, repeats, 1) as _i:
                    body(_i)

    nc.compile()
    return nc


def _prep_in_maps(x, rope_cos, rope_sin, W_qkv, b_qkv, W_out, b_out,
                  with_bias=False):
    f32 = np.float32
    W3 = np.asarray(W_qkv, dtype=f32).reshape(D, 16, 3, HD)  # [f, head, qkv, d]
    b3 = np.asarray(b_qkv, dtype=f32).reshape(16, 3, HD)
    cos_r = np.ascontiguousarray(np.asarray(rope_cos, dtype=f32))
    sin_r = np.ascontiguousarray(np.asarray(rope_sin, dtype=f32))
    ones = np.ones((1, 128), dtype=f32)
    onescol = np.ones((128, 64), dtype=f32)
    W_out = np.asarray(W_out, dtype=f32)
    b_out = np.asarray(b_out, dtype=f32)
    x = np.asarray(x, dtype=f32)

    in_maps = []
    for c in range(N_CORES):
        b, hg = divmod(c, 4)
        hs = slice(hg * H_LOC, (hg + 1) * H_LOC)
        wq = W3[:, hs, 0, :].reshape(D, 256)
        wk = W3[:, hs, 1, :].reshape(D, 256)
        wv = W3[:, hs, 2, :].reshape(D, 256)
        import ml_dtypes
        m = {
            "xt": np.ascontiguousarray(x[b].T).astype(ml_dtypes.bfloat16),
            "cosr": cos_r, "sinr": sin_r,
            "wqk": np.ascontiguousarray(
                np.concatenate([wq, wk], axis=1)).astype(ml_dtypes.bfloat16),
            "wv": np.ascontiguousarray(wv).astype(ml_dtypes.bfloat16),
            "wout": np.ascontiguousarray(W_out[hg * 256:(hg + 1) * 256, :]),
            "ones": ones, "onescol": onescol,
            "ident": np.eye(128, dtype=f32),
        }
        if with_bias:
            bq = b3[hs, 0, :].reshape(1, 256)
            bk = b3[hs, 1, :].reshape(1, 256)
            m["bqk"] = np.ascontiguousarray(np.concatenate([bq, bk], axis=1))
            m["bv"] = np.ascontiguousarray(b3[hs, 2, :].reshape(1, 256))
            m["bout"] = (np.ascontiguousarray(b_out.reshape(1, D)) if hg == 0
                         else np.zeros((1, D), dtype=f32))
        in_maps.append(m)
    return in_maps


def kernel(x, rope_cos, rope_sin, W_qkv, b_qkv, W_out, b_out):
    from concourse.bass_utils import run_bass_kernel_spmd

    with_bias = bool(np.any(np.asarray(b_qkv)) or np.any(np.asarray(b_out)))
    key = ("nc", with_bias)
    if key not in _CACHED:
        _CACHED[key] = build_nc(1, with_bias=with_bias)
        _CACHED["nc"] = _CACHED[key]  # convenience for test harness
    nc = _CACHED[key]
    in_maps = _prep_in_maps(x, rope_cos, rope_sin, W_qkv, b_qkv, W_out, b_out,
                            with_bias=with_bias)
    res = run_bass_kernel_spmd(nc, in_maps, list(range(N_CORES)))
    B = x.shape[0]
    out = np.zeros((B, S, D), dtype=np.float32)
    for c in range(N_CORES):
        b = c // 4
        out[b] += res.results[c]["y"].astype(np.float32)
    return out
